# revision 2
# baseline (speedup 1.0000x reference)
"""CoordinateDensification kernel for 8 TRN2 NeuronCores.

Reference semantics: expand 500k int32 coords [N,4] (cols 0-2 in [0,256),
col 3 == 0) by the 27 offsets {-2,0,2}^3 (stride 2), then sorted row-dedup
padded with INT32_MAX to [N*27, 4].

Device algorithm (SPMD over 8 cores, sharded by z-slab):
  - occupancy grid per core: 37 z-planes (33 owned + 2 halo each side) of
    260y x 512x bytes; coords scattered via GPSIMD indirect DMA.
  - 3D binary dilation by {-2,0,2}^3: z/y via shifted plane loads OR'd on
    DVE, x via shifted free-dim ORs.
  - outputs the dilated bitmask (33 planes / core).
Host: bins coords per core (sharding), then flatnonzero + unpack + pad
(gather/unshard). Bitmask cell order == lexicographic row order of the
reference output, so no sort is ever needed.
"""
import sys
sys.path.insert(0, '/opt/trn_rl_repo')
import numpy as np

N = 500000
ZPL = 33               # dilated planes owned per core
GRIDP = ZPL + 4        # occupancy planes incl. halo
PLANE = 260 * 512      # bytes per plane (x padded 260->512)
GRID_CELLS = GRIDP * PLANE
IDX_COLS = 624
NIDX = IDX_COLS * 128  # padded coord-index capacity per core
FILL = np.int32(np.iinfo(np.int32).max)
OUT_ROWS = N * 27

_NC_CACHE = {}


def _build_nc():
    if "nc" in _NC_CACHE:
        return _NC_CACHE["nc"]
    import concourse.bass as bass
    import concourse.bacc as bacc
    import concourse.tile as tile
    from concourse import mybir

    u8 = mybir.dt.uint8
    i32 = mybir.dt.int32
    MAX = mybir.AluOpType.max

    nc = bacc.Bacc("TRN2", target_bir_lowering=False, num_devices=8)
    idxin = nc.dram_tensor("idxin", [128, IDX_COLS], i32, kind="ExternalInput")
    dil = nc.dram_tensor("dil", [ZPL * 260, 512], u8, kind="ExternalOutput")
    grid = nc.dram_tensor("grid", [GRID_CELLS, 1], u8)

    with tile.TileContext(nc) as tc:
        with (
            tc.tile_pool(name="sbuf", bufs=2) as pool,
            tc.tile_pool(name="ld", bufs=8) as ldp,
        ):
            # ---- zero the occupancy grid ----
            z8 = pool.tile([128, 8192], u8, tag="z8")
            nc.vector.memset(z8[:], 0)
            CH = 128 * 8192
            nfull = GRID_CELLS // CH
            for i in range(nfull):
                nc.sync.dma_start(
                    out=bass.AP(grid, i * CH, [[8192, 128], [1, 8192]]),
                    in_=z8[:],
                )
            remc = (GRID_CELLS - nfull * CH) // 128
            nc.sync.dma_start(
                out=bass.AP(grid, nfull * CH, [[remc, 128], [1, remc]]),
                in_=z8[:, :remc],
            )
            # ---- load indices, scatter occupancy ----
            idxsb = pool.tile([128, IDX_COLS], i32, tag="idx")
            nc.scalar.dma_start(out=idxsb[:], in_=idxin[:, :])
            ones = pool.tile([128, IDX_COLS], u8, tag="ones")
            nc.vector.memset(ones[:], 1)
            tc.strict_bb_all_engine_barrier()
            for w in range(IDX_COLS):
                nc.gpsimd.indirect_dma_start(
                    out=grid[:, :],
                    out_offset=bass.IndirectOffsetOnAxis(ap=idxsb[:, w:w + 1], axis=0),
                    in_=ones[:, w:w + 1],
                    in_offset=None,
                    bounds_check=GRID_CELLS - 1,
                    oob_is_err=False,
                )
            tc.strict_bb_all_engine_barrier()
            # ---- dilation ----
            engs = [nc.sync, nc.scalar]
            li = 0
            for zi in range(ZPL):
                for (r0, nrows) in ((0, 128), (128, 128), (256, 4)):
                    acc = ldp.tile([128, 512], u8, tag="acc")
                    first = True
                    for dz in (0, 2, 4):
                        for dy in (-2, 0, 2):
                            rs = r0 + dy
                            s = max(0, rs)
                            e = min(260, rs + nrows)
                            tmp = ldp.tile([128, 512], u8, tag=f"tmp{li % 4}")
                            if e - s < nrows:
                                nc.vector.memset(tmp[:nrows], 0)
                            off = (zi + dz) * PLANE + s * 512
                            engs[li % 2].dma_start(
                                out=tmp[s - rs:s - rs + (e - s), :],
                                in_=bass.AP(grid, off, [[512, e - s], [1, 512]]),
                            )
                            li += 1
                            if first:
                                nc.vector.tensor_copy(acc[:nrows], tmp[:nrows])
                                first = False
                            else:
                                nc.vector.tensor_tensor(
                                    out=acc[:nrows], in0=acc[:nrows],
                                    in1=tmp[:nrows], op=MAX)
                    fin = ldp.tile([128, 512], u8, tag="fin")
                    nc.vector.tensor_copy(fin[:nrows], acc[:nrows])
                    nc.vector.tensor_tensor(
                        out=fin[:nrows, 0:510], in0=fin[:nrows, 0:510],
                        in1=acc[:nrows, 2:512], op=MAX)
                    nc.vector.tensor_tensor(
                        out=fin[:nrows, 2:512], in0=fin[:nrows, 2:512],
                        in1=acc[:nrows, 0:510], op=MAX)
                    nc.sync.dma_start(
                        out=dil[zi * 260 + r0: zi * 260 + r0 + nrows, :],
                        in_=fin[:nrows, :],
                    )
    nc.compile()
    _NC_CACHE["nc"] = nc
    return nc


def _shard_inputs(coords):
    zp = coords[:, 0].astype(np.int64) + 2
    yp = coords[:, 1].astype(np.int64) + 2
    xp = coords[:, 2].astype(np.int64) + 2
    in_maps = []
    for c in range(8):
        lo = 33 * c - 2
        sel = (zp >= lo) & (zp < lo + GRIDP)
        idx = ((zp[sel] - lo) * PLANE + yp[sel] * 512 + xp[sel]).astype(np.int32)
        if idx.size > NIDX:
            raise ValueError(f"core {c}: {idx.size} coords exceed capacity {NIDX}")
        pad = np.full(NIDX, 0x7FFF0000, np.int32)
        pad[:idx.size] = idx
        in_maps.append({"idxin": np.ascontiguousarray(pad.reshape(IDX_COLS, 128).T)})
    return in_maps


def kernel(coords, stride):
    from concourse.bass_utils import run_bass_kernel_spmd

    coords = np.asarray(coords)
    stride = int(np.asarray(stride))
    assert stride == 2, f"kernel hardcodes stride 2, got {stride}"
    assert coords.shape == (N, 4)

    nc = _build_nc()
    in_maps = _shard_inputs(coords)
    res = run_bass_kernel_spmd(nc, in_maps, core_ids=list(range(8)))

    keys = []
    for c in range(8):
        npl = min(ZPL, 260 - ZPL * c)
        m = np.asarray(res.results[c]["dil"]).reshape(-1)[: npl * PLANE]
        keys.append(np.flatnonzero(m) + ZPL * c * PLANE)
    keys = np.concatenate(keys)
    total = keys.size
    out = np.full((OUT_ROWS, 4), FILL, np.int32)
    zq = keys // PLANE
    rem = keys - zq * PLANE
    out[:total, 0] = zq - 2
    out[:total, 1] = (rem >> 9) - 2
    out[:total, 2] = (rem & 511) - 2
    out[:total, 3] = 0
    return out


# revision 5
# speedup vs baseline: 2.3045x; 2.3045x over previous
"""CoordinateDensification kernel for 8 TRN2 NeuronCores.

Reference semantics: expand 500k int32 coords [N,4] (cols 0-2 in [0,256),
col 3 == 0) by the 27 offsets {-2,0,2}^3 (stride 2), then sorted row-dedup
padded with INT32_MAX to [N*27, 4].

Device algorithm (SPMD over 8 cores, sharded by z-slab):
  - occupancy grid per core: 37 z-planes (33 owned + 2 halo each side) of
    260y x 512x bytes; coords scattered via GPSIMD indirect DMA.
  - 3D binary dilation by {-2,0,2}^3: z/y via shifted plane loads OR'd on
    DVE, x via shifted free-dim ORs.
  - outputs the dilated bitmask (33 planes / core).
Host: bins coords per core (sharding), then flatnonzero + unpack + pad
(gather/unshard). Bitmask cell order == lexicographic row order of the
reference output, so no sort is ever needed.
"""
import sys
sys.path.insert(0, '/opt/trn_rl_repo')
import numpy as np

N = 500000
ZPL = 33               # dilated planes owned per core
GRIDP = ZPL + 4        # occupancy planes incl. halo
PLANE = 260 * 512      # bytes per plane (x padded 260->512)
GRID_CELLS = GRIDP * PLANE
IDX_COLS = 624
NIDX = IDX_COLS * 128  # padded coord-index capacity per core
FILL = np.int32(np.iinfo(np.int32).max)
OUT_ROWS = N * 27

_NC_CACHE = {}


def _build_nc():
    if "nc" in _NC_CACHE:
        return _NC_CACHE["nc"]
    import concourse.bass as bass
    import concourse.bacc as bacc
    import concourse.tile as tile
    from concourse import mybir

    u8 = mybir.dt.uint8
    i32 = mybir.dt.int32
    MAX = mybir.AluOpType.max

    MUL = mybir.AluOpType.mult
    ADD = mybir.AluOpType.add
    nc = bacc.Bacc("TRN2", target_bir_lowering=False, num_devices=8)
    idxin = nc.dram_tensor("idxin", [128, IDX_COLS], i32, kind="ExternalInput")
    dil = nc.dram_tensor("dil", [ZPL * 260, 33], u8, kind="ExternalOutput")
    grid = nc.dram_tensor("grid", [GRID_CELLS, 1], u8)

    with tile.TileContext(nc) as tc:
        with (
            tc.tile_pool(name="sbuf", bufs=2) as pool,
            tc.tile_pool(name="ld", bufs=8) as ldp,
        ):
            # ---- zero the occupancy grid ----
            z8 = pool.tile([128, 8192], u8, tag="z8")
            nc.vector.memset(z8[:], 0)
            CH = 128 * 8192
            nfull = GRID_CELLS // CH
            for i in range(nfull):
                nc.sync.dma_start(
                    out=bass.AP(grid, i * CH, [[8192, 128], [1, 8192]]),
                    in_=z8[:],
                )
            remc = (GRID_CELLS - nfull * CH) // 128
            nc.sync.dma_start(
                out=bass.AP(grid, nfull * CH, [[remc, 128], [1, remc]]),
                in_=z8[:, :remc],
            )
            # ---- load indices, scatter occupancy ----
            idxsb = pool.tile([128, IDX_COLS], i32, tag="idx")
            nc.scalar.dma_start(out=idxsb[:], in_=idxin[:, :])
            ones = pool.tile([128, IDX_COLS], u8, tag="ones")
            nc.vector.memset(ones[:], 1)
            tc.strict_bb_all_engine_barrier()
            for w in range(IDX_COLS):
                nc.gpsimd.indirect_dma_start(
                    out=grid[:, :],
                    out_offset=bass.IndirectOffsetOnAxis(ap=idxsb[:, w:w + 1], axis=0),
                    in_=ones[:, w:w + 1],
                    in_offset=None,
                    bounds_check=GRID_CELLS - 1,
                    oob_is_err=False,
                )
            tc.strict_bb_all_engine_barrier()
            # ---- dilation ----
            engs = [nc.sync, nc.scalar]
            li = 0
            for zi in range(ZPL):
                for (r0, nrows) in ((0, 128), (128, 128), (256, 4)):
                    acc = ldp.tile([128, 512], u8, tag="acc")
                    first = True
                    for dz in (0, 2, 4):
                        for dy in (-2, 0, 2):
                            rs = r0 + dy
                            s = max(0, rs)
                            e = min(260, rs + nrows)
                            tmp = ldp.tile([128, 512], u8, tag=f"tmp{li % 4}")
                            if e - s < nrows:
                                nc.vector.memset(tmp[:nrows], 0)
                            off = (zi + dz) * PLANE + s * 512
                            engs[li % 2].dma_start(
                                out=tmp[s - rs:s - rs + (e - s), :],
                                in_=bass.AP(grid, off, [[512, e - s], [1, 512]]),
                            )
                            li += 1
                            if first:
                                nc.vector.tensor_copy(acc[:nrows], tmp[:nrows])
                                first = False
                            else:
                                nc.vector.tensor_tensor(
                                    out=acc[:nrows], in0=acc[:nrows],
                                    in1=tmp[:nrows], op=MAX)
                    fin = ldp.tile([128, 512], u8, tag="fin")
                    nc.vector.tensor_copy(fin[:nrows], acc[:nrows])
                    nc.vector.tensor_tensor(
                        out=fin[:nrows, 0:510], in0=fin[:nrows, 0:510],
                        in1=acc[:nrows, 2:512], op=MAX)
                    nc.vector.tensor_tensor(
                        out=fin[:nrows, 2:512], in0=fin[:nrows, 2:512],
                        in1=acc[:nrows, 0:510], op=MAX)
                    # bit-pack cells 0..263 -> 33 bytes/row (little bit order)
                    p1 = ldp.tile([128, 132], u8, tag="p1")
                    nc.vector.scalar_tensor_tensor(
                        out=p1[:nrows], in0=fin[:nrows, 1:264:2], scalar=2,
                        in1=fin[:nrows, 0:264:2], op0=MUL, op1=ADD)
                    p2 = ldp.tile([128, 66], u8, tag="p2")
                    nc.vector.scalar_tensor_tensor(
                        out=p2[:nrows], in0=p1[:nrows, 1:132:2], scalar=4,
                        in1=p1[:nrows, 0:132:2], op0=MUL, op1=ADD)
                    p3 = ldp.tile([128, 33], u8, tag="p3")
                    nc.vector.scalar_tensor_tensor(
                        out=p3[:nrows], in0=p2[:nrows, 1:66:2], scalar=16,
                        in1=p2[:nrows, 0:66:2], op0=MUL, op1=ADD)
                    nc.sync.dma_start(
                        out=dil[zi * 260 + r0: zi * 260 + r0 + nrows, :],
                        in_=p3[:nrows, :],
                    )
    nc.compile()
    _NC_CACHE["nc"] = nc
    return nc


def _shard_inputs(coords):
    zp = coords[:, 0].astype(np.int64) + 2
    yp = coords[:, 1].astype(np.int64) + 2
    xp = coords[:, 2].astype(np.int64) + 2
    in_maps = []
    for c in range(8):
        lo = 33 * c - 2
        sel = (zp >= lo) & (zp < lo + GRIDP)
        idx = ((zp[sel] - lo) * PLANE + yp[sel] * 512 + xp[sel]).astype(np.int32)
        if idx.size > NIDX:
            raise ValueError(f"core {c}: {idx.size} coords exceed capacity {NIDX}")
        pad = np.full(NIDX, 0x7FFF0000, np.int32)
        pad[:idx.size] = idx
        in_maps.append({"idxin": np.ascontiguousarray(pad.reshape(IDX_COLS, 128).T)})
    return in_maps


def kernel(coords, stride):
    from concourse.bass_utils import run_bass_kernel_spmd

    coords = np.asarray(coords)
    stride = int(np.asarray(stride))
    assert stride == 2, f"kernel hardcodes stride 2, got {stride}"
    assert coords.shape == (N, 4)

    nc = _build_nc()
    in_maps = _shard_inputs(coords)
    res = run_bass_kernel_spmd(nc, in_maps, core_ids=list(range(8)))

    keys = []
    for c in range(8):
        npl = min(ZPL, 260 - ZPL * c)
        packed = np.asarray(res.results[c]["dil"])[: npl * 260, :]
        bits = np.unpackbits(packed, axis=1, bitorder="little")[:, :260]
        m = np.ascontiguousarray(bits).reshape(-1)
        keys.append(np.flatnonzero(m) + ZPL * c * (260 * 260))
    keys = np.concatenate(keys)
    total = keys.size
    out = np.full((OUT_ROWS, 4), FILL, np.int32)
    r, x = np.divmod(keys, 260)
    zq, y = np.divmod(r, 260)
    out[:total, 0] = zq - 2
    out[:total, 1] = y - 2
    out[:total, 2] = x - 2
    out[:total, 3] = 0
    return out


# revision 9
# speedup vs baseline: 2.3444x; 1.0173x over previous
"""CoordinateDensification kernel for 8 TRN2 NeuronCores.

Reference semantics: expand 500k int32 coords [N,4] (cols 0-2 in [0,256),
col 3 == 0) by the 27 offsets {-2,0,2}^3 (stride 2), then sorted row-dedup
padded with INT32_MAX to [N*27, 4].

Device algorithm (SPMD over 8 cores, sharded by z-slab):
  - occupancy grid per core: 37 z-planes (33 owned + 2 halo each side) of
    260y x 512x bytes; coords scattered via GPSIMD indirect DMA.
  - 3D binary dilation by {-2,0,2}^3: z/y via shifted plane loads OR'd on
    DVE, x via shifted free-dim ORs.
  - outputs the dilated bitmask (33 planes / core).
Host: bins coords per core (sharding), then flatnonzero + unpack + pad
(gather/unshard). Bitmask cell order == lexicographic row order of the
reference output, so no sort is ever needed.
"""
import sys
sys.path.insert(0, '/opt/trn_rl_repo')
import numpy as np

N = 500000
ZPL = 33               # dilated planes owned per core
GRIDP = ZPL + 4        # occupancy planes incl. halo
PLANE = 260 * 512      # bytes per plane (x padded 260->512)
GRID_CELLS = GRIDP * PLANE
IDX_COLS = 624
NIDX = IDX_COLS * 128  # padded coord-index capacity per core
FILL = np.int32(np.iinfo(np.int32).max)
OUT_ROWS = N * 27

_NC_CACHE = {}


def _build_nc():
    if "nc" in _NC_CACHE:
        return _NC_CACHE["nc"]
    import concourse.bass as bass
    import concourse.bacc as bacc
    import concourse.tile as tile
    from concourse import mybir

    u8 = mybir.dt.uint8
    i32 = mybir.dt.int32
    MAX = mybir.AluOpType.max

    MUL = mybir.AluOpType.mult
    ADD = mybir.AluOpType.add
    nc = bacc.Bacc("TRN2", target_bir_lowering=False, num_devices=8)
    idxin = nc.dram_tensor("idxin", [128, IDX_COLS], i32, kind="ExternalInput")
    dil = nc.dram_tensor("dil", [ZPL * 260, 33], u8, kind="ExternalOutput")
    grid = nc.dram_tensor("grid", [GRID_CELLS, 1], u8)

    with tile.TileContext(nc) as tc:
        with (
            tc.tile_pool(name="sbuf", bufs=2) as pool,
            tc.tile_pool(name="ld", bufs=8) as ldp,
        ):
            # ---- zero the occupancy grid ----
            z8 = pool.tile([128, 8192], u8, tag="z8")
            nc.vector.memset(z8[:], 0)
            CH = 128 * 8192
            nfull = GRID_CELLS // CH
            for i in range(nfull):
                nc.sync.dma_start(
                    out=bass.AP(grid, i * CH, [[8192, 128], [1, 8192]]),
                    in_=z8[:],
                )
            remc = (GRID_CELLS - nfull * CH) // 128
            nc.sync.dma_start(
                out=bass.AP(grid, nfull * CH, [[remc, 128], [1, remc]]),
                in_=z8[:, :remc],
            )
            # ---- load indices, scatter occupancy ----
            idxsb = pool.tile([128, IDX_COLS], i32, tag="idx")
            nc.scalar.dma_start(out=idxsb[:], in_=idxin[:, :])
            ones = pool.tile([128, IDX_COLS], u8, tag="ones")
            nc.vector.memset(ones[:], 1)
            tc.strict_bb_all_engine_barrier()
            SW = 1  # descriptor batch width per indirect DMA (SW>1 mis-pairs offsets)
            for w in range(0, IDX_COLS, SW):
                nc.gpsimd.indirect_dma_start(
                    out=grid[:, :],
                    out_offset=bass.IndirectOffsetOnAxis(ap=idxsb[:, w:w + SW], axis=0),
                    in_=ones[:, w:w + SW],
                    in_offset=None,
                    bounds_check=GRID_CELLS - 1,
                    oob_is_err=False,
                )
            tc.strict_bb_all_engine_barrier()
            # ---- dilation ----
            engs = [nc.sync, nc.scalar]
            li = 0
            for zi in range(ZPL):
                for (r0, nrows) in ((0, 128), (128, 128), (256, 4)):
                    acc = ldp.tile([128, 512], u8, tag="acc")
                    first = True
                    for dz in (0, 2, 4):
                        for dy in (-2, 0, 2):
                            rs = r0 + dy
                            s = max(0, rs)
                            e = min(260, rs + nrows)
                            tmp = ldp.tile([128, 512], u8, tag=f"tmp{li % 4}")
                            if e - s < nrows:
                                nc.vector.memset(tmp[:nrows], 0)
                            off = (zi + dz) * PLANE + s * 512
                            engs[li % 2].dma_start(
                                out=tmp[s - rs:s - rs + (e - s), :],
                                in_=bass.AP(grid, off, [[512, e - s], [1, 512]]),
                            )
                            li += 1
                            if first:
                                nc.vector.tensor_copy(acc[:nrows], tmp[:nrows])
                                first = False
                            else:
                                nc.vector.tensor_tensor(
                                    out=acc[:nrows], in0=acc[:nrows],
                                    in1=tmp[:nrows], op=MAX)
                    fin = ldp.tile([128, 512], u8, tag="fin")
                    nc.vector.tensor_copy(fin[:nrows], acc[:nrows])
                    nc.vector.tensor_tensor(
                        out=fin[:nrows, 0:510], in0=fin[:nrows, 0:510],
                        in1=acc[:nrows, 2:512], op=MAX)
                    nc.vector.tensor_tensor(
                        out=fin[:nrows, 2:512], in0=fin[:nrows, 2:512],
                        in1=acc[:nrows, 0:510], op=MAX)
                    # bit-pack cells 0..263 -> 33 bytes/row (little bit order)
                    p1 = ldp.tile([128, 132], u8, tag="p1")
                    nc.vector.scalar_tensor_tensor(
                        out=p1[:nrows], in0=fin[:nrows, 1:264:2], scalar=2,
                        in1=fin[:nrows, 0:264:2], op0=MUL, op1=ADD)
                    p2 = ldp.tile([128, 66], u8, tag="p2")
                    nc.vector.scalar_tensor_tensor(
                        out=p2[:nrows], in0=p1[:nrows, 1:132:2], scalar=4,
                        in1=p1[:nrows, 0:132:2], op0=MUL, op1=ADD)
                    p3 = ldp.tile([128, 33], u8, tag="p3")
                    nc.vector.scalar_tensor_tensor(
                        out=p3[:nrows], in0=p2[:nrows, 1:66:2], scalar=16,
                        in1=p2[:nrows, 0:66:2], op0=MUL, op1=ADD)
                    nc.sync.dma_start(
                        out=dil[zi * 260 + r0: zi * 260 + r0 + nrows, :],
                        in_=p3[:nrows, :],
                    )
    nc.compile()
    _NC_CACHE["nc"] = nc
    return nc


def _shard_inputs(coords):
    zp = coords[:, 0].astype(np.int64) + 2
    yp = coords[:, 1].astype(np.int64) + 2
    xp = coords[:, 2].astype(np.int64) + 2
    in_maps = []
    for c in range(8):
        lo = 33 * c - 2
        sel = (zp >= lo) & (zp < lo + GRIDP)
        idx = ((zp[sel] - lo) * PLANE + yp[sel] * 512 + xp[sel]).astype(np.int32)
        if idx.size > NIDX:
            raise ValueError(f"core {c}: {idx.size} coords exceed capacity {NIDX}")
        pad = np.full(NIDX, 0x7FFF0000, np.int32)
        pad[:idx.size] = idx
        in_maps.append({"idxin": np.ascontiguousarray(pad.reshape(IDX_COLS, 128).T)})
    return in_maps


def kernel(coords, stride):
    from concourse.bass_utils import run_bass_kernel_spmd

    coords = np.asarray(coords)
    stride = int(np.asarray(stride))
    assert stride == 2, f"kernel hardcodes stride 2, got {stride}"
    assert coords.shape == (N, 4)

    nc = _build_nc()
    in_maps = _shard_inputs(coords)
    res = run_bass_kernel_spmd(nc, in_maps, core_ids=list(range(8)))

    keys = []
    for c in range(8):
        npl = min(ZPL, 260 - ZPL * c)
        packed = np.asarray(res.results[c]["dil"])[: npl * 260, :]
        bits = np.unpackbits(packed, axis=1, bitorder="little")[:, :260]
        m = np.ascontiguousarray(bits).reshape(-1)
        keys.append(np.flatnonzero(m) + ZPL * c * (260 * 260))
    keys = np.concatenate(keys).astype(np.int32)
    total = keys.size
    out = np.empty((OUT_ROWS, 4), np.int32)
    r, x = np.divmod(keys, np.int32(260))
    zq, y = np.divmod(r, np.int32(260))
    body = out[:total]
    body[:, 0] = zq
    body[:, 1] = y
    body[:, 2] = x
    body[:, 0:3] -= np.int32(2)
    body[:, 3] = 0
    out[total:] = FILL
    return out


# revision 10
# speedup vs baseline: 8.4006x; 3.5833x over previous
"""CoordinateDensification kernel for 8 TRN2 NeuronCores.

Reference semantics: expand 500k int32 coords [N,4] (cols 0-2 in [0,256),
col 3 == 0) by the 27 offsets {-2,0,2}^3 (stride 2), then sorted row-dedup
padded with INT32_MAX to [N*27, 4].

Device algorithm (SPMD over 8 cores, sharded by z-slab):
  - occupancy grid per core: 37 z-planes (33 owned + 2 halo each side) of
    260y x 512x bytes; coords scattered via GPSIMD indirect DMA.
  - 3D binary dilation by {-2,0,2}^3: z/y via shifted plane loads OR'd on
    DVE, x via shifted free-dim ORs.
  - outputs the dilated bitmask (33 planes / core).
Host: bins coords per core (sharding), then flatnonzero + unpack + pad
(gather/unshard). Bitmask cell order == lexicographic row order of the
reference output, so no sort is ever needed.
"""
import sys
sys.path.insert(0, '/opt/trn_rl_repo')
import numpy as np

N = 500000
ZPL = 33               # dilated planes owned per core
GRIDP = ZPL + 4        # occupancy planes incl. halo
PLANE = 260 * 512      # bytes per plane (x padded 260->512)
GRID_CELLS = GRIDP * PLANE
IDX_COLS = 624
NIDX = IDX_COLS * 128  # padded coord-index capacity per core
FILL = np.int32(np.iinfo(np.int32).max)
OUT_ROWS = N * 27

_NC_CACHE = {}


def _build_nc():
    if "nc" in _NC_CACHE:
        return _NC_CACHE["nc"]
    import concourse.bass as bass
    import concourse.bacc as bacc
    import concourse.tile as tile
    from concourse import mybir

    u8 = mybir.dt.uint8
    i32 = mybir.dt.int32
    MAX = mybir.AluOpType.max

    MUL = mybir.AluOpType.mult
    ADD = mybir.AluOpType.add
    nc = bacc.Bacc("TRN2", target_bir_lowering=False, num_devices=8)
    idxin = nc.dram_tensor("idxin", [128, IDX_COLS], i32, kind="ExternalInput")
    dil = nc.dram_tensor("dil", [ZPL * 260, 33], u8, kind="ExternalOutput")
    grid = nc.dram_tensor("grid", [GRID_CELLS, 1], u8)

    with tile.TileContext(nc) as tc:
        with (
            tc.tile_pool(name="sbuf", bufs=2) as pool,
            tc.tile_pool(name="ld", bufs=8) as ldp,
        ):
            # ---- zero the occupancy grid ----
            z8 = pool.tile([128, 8192], u8, tag="z8")
            nc.vector.memset(z8[:], 0)
            CH = 128 * 8192
            nfull = GRID_CELLS // CH
            for i in range(nfull):
                nc.sync.dma_start(
                    out=bass.AP(grid, i * CH, [[8192, 128], [1, 8192]]),
                    in_=z8[:],
                )
            remc = (GRID_CELLS - nfull * CH) // 128
            nc.sync.dma_start(
                out=bass.AP(grid, nfull * CH, [[remc, 128], [1, remc]]),
                in_=z8[:, :remc],
            )
            # ---- load indices, scatter occupancy ----
            idxsb = pool.tile([128, IDX_COLS], i32, tag="idx")
            nc.scalar.dma_start(out=idxsb[:], in_=idxin[:, :])
            ones = pool.tile([128, IDX_COLS], u8, tag="ones")
            nc.vector.memset(ones[:], 1)
            tc.strict_bb_all_engine_barrier()
            SW = 1  # descriptor batch width per indirect DMA (SW>1 mis-pairs offsets)
            for w in range(0, IDX_COLS, SW):
                nc.gpsimd.indirect_dma_start(
                    out=grid[:, :],
                    out_offset=bass.IndirectOffsetOnAxis(ap=idxsb[:, w:w + SW], axis=0),
                    in_=ones[:, w:w + SW],
                    in_offset=None,
                    bounds_check=GRID_CELLS - 1,
                    oob_is_err=False,
                )
            tc.strict_bb_all_engine_barrier()
            # ---- dilation ----
            engs = [nc.sync, nc.scalar]
            li = 0
            for zi in range(ZPL):
                for (r0, nrows) in ((0, 128), (128, 128), (256, 4)):
                    acc = ldp.tile([128, 512], u8, tag="acc")
                    first = True
                    for dz in (0, 2, 4):
                        for dy in (-2, 0, 2):
                            rs = r0 + dy
                            s = max(0, rs)
                            e = min(260, rs + nrows)
                            tmp = ldp.tile([128, 512], u8, tag=f"tmp{li % 4}")
                            if e - s < nrows:
                                nc.vector.memset(tmp[:nrows], 0)
                            off = (zi + dz) * PLANE + s * 512
                            engs[li % 2].dma_start(
                                out=tmp[s - rs:s - rs + (e - s), :],
                                in_=bass.AP(grid, off, [[512, e - s], [1, 512]]),
                            )
                            li += 1
                            if first:
                                nc.vector.tensor_copy(acc[:nrows], tmp[:nrows])
                                first = False
                            else:
                                nc.vector.tensor_tensor(
                                    out=acc[:nrows], in0=acc[:nrows],
                                    in1=tmp[:nrows], op=MAX)
                    fin = ldp.tile([128, 512], u8, tag="fin")
                    nc.vector.tensor_copy(fin[:nrows], acc[:nrows])
                    nc.vector.tensor_tensor(
                        out=fin[:nrows, 0:510], in0=fin[:nrows, 0:510],
                        in1=acc[:nrows, 2:512], op=MAX)
                    nc.vector.tensor_tensor(
                        out=fin[:nrows, 2:512], in0=fin[:nrows, 2:512],
                        in1=acc[:nrows, 0:510], op=MAX)
                    # bit-pack cells 0..263 -> 33 bytes/row (little bit order)
                    p1 = ldp.tile([128, 132], u8, tag="p1")
                    nc.vector.scalar_tensor_tensor(
                        out=p1[:nrows], in0=fin[:nrows, 1:264:2], scalar=2,
                        in1=fin[:nrows, 0:264:2], op0=MUL, op1=ADD)
                    p2 = ldp.tile([128, 66], u8, tag="p2")
                    nc.vector.scalar_tensor_tensor(
                        out=p2[:nrows], in0=p1[:nrows, 1:132:2], scalar=4,
                        in1=p1[:nrows, 0:132:2], op0=MUL, op1=ADD)
                    p3 = ldp.tile([128, 33], u8, tag="p3")
                    nc.vector.scalar_tensor_tensor(
                        out=p3[:nrows], in0=p2[:nrows, 1:66:2], scalar=16,
                        in1=p2[:nrows, 0:66:2], op0=MUL, op1=ADD)
                    nc.sync.dma_start(
                        out=dil[zi * 260 + r0: zi * 260 + r0 + nrows, :],
                        in_=p3[:nrows, :],
                    )
    nc.compile()
    _NC_CACHE["nc"] = nc
    return nc


def _shard_inputs(coords):
    zp = coords[:, 0].astype(np.int64) + 2
    yp = coords[:, 1].astype(np.int64) + 2
    xp = coords[:, 2].astype(np.int64) + 2
    in_maps = []
    for c in range(8):
        lo = 33 * c - 2
        sel = (zp >= lo) & (zp < lo + GRIDP)
        idx = ((zp[sel] - lo) * PLANE + yp[sel] * 512 + xp[sel]).astype(np.int32)
        if idx.size > NIDX:
            raise ValueError(f"core {c}: {idx.size} coords exceed capacity {NIDX}")
        pad = np.full(NIDX, 0x7FFF0000, np.int32)
        pad[:idx.size] = idx
        in_maps.append({"idxin": np.ascontiguousarray(pad.reshape(IDX_COLS, 128).T)})
    return in_maps


_LAST_TIMES = {}


def kernel(coords, stride):
    import time as _time
    from concourse.bass_utils import run_bass_kernel_spmd

    coords = np.asarray(coords)
    stride = int(np.asarray(stride))
    assert stride == 2, f"kernel hardcodes stride 2, got {stride}"
    assert coords.shape == (N, 4)

    t0 = _time.time()
    nc = _build_nc()
    t1 = _time.time()
    in_maps = _shard_inputs(coords)
    t2 = _time.time()
    res = run_bass_kernel_spmd(nc, in_maps, core_ids=list(range(8)))
    t3 = _time.time()
    _LAST_TIMES.update(build=t1 - t0, shard=t2 - t1, device=t3 - t2)

    keys = []
    for c in range(8):
        npl = min(ZPL, 260 - ZPL * c)
        packed = np.asarray(res.results[c]["dil"])[: npl * 260, :]
        bits = np.unpackbits(packed, axis=1, bitorder="little")[:, :260]
        m = np.ascontiguousarray(bits).reshape(-1)
        keys.append(np.flatnonzero(m) + ZPL * c * (260 * 260))
    keys = np.concatenate(keys).astype(np.int32)
    total = keys.size
    out = np.empty((OUT_ROWS, 4), np.int32)
    r, x = np.divmod(keys, np.int32(260))
    zq, y = np.divmod(r, np.int32(260))
    body = out[:total]
    body[:, 0] = zq
    body[:, 1] = y
    body[:, 2] = x
    body[:, 0:3] -= np.int32(2)
    body[:, 3] = 0
    out[total:] = FILL
    return out


# revision 11
# speedup vs baseline: 10.5577x; 1.2568x over previous
"""CoordinateDensification kernel for 8 TRN2 NeuronCores.

Reference semantics: expand 500k int32 coords [N,4] (cols 0-2 in [0,256),
col 3 == 0) by the 27 offsets {-2,0,2}^3 (stride 2), then sorted row-dedup
padded with INT32_MAX to [N*27, 4].

Device algorithm (SPMD over 8 cores, sharded by z-slab):
  - occupancy grid per core: 37 z-planes (33 owned + 2 halo each side) of
    260y x 512x bytes; coords scattered via GPSIMD indirect DMA.
  - 3D binary dilation by {-2,0,2}^3: z/y via shifted plane loads OR'd on
    DVE, x via shifted free-dim ORs.
  - outputs the dilated bitmask (33 planes / core).
Host: bins coords per core (sharding), then flatnonzero + unpack + pad
(gather/unshard). Bitmask cell order == lexicographic row order of the
reference output, so no sort is ever needed.
"""
import sys
sys.path.insert(0, '/opt/trn_rl_repo')
import numpy as np

N = 500000
ZPL = 33               # dilated planes owned per core
GRIDP = ZPL + 4        # occupancy planes incl. halo
PLANE = 260 * 512      # bytes per plane (x padded 260->512)
GRID_CELLS = GRIDP * PLANE
IDX_COLS = 624
NIDX = IDX_COLS * 128  # padded coord-index capacity per core
FILL = np.int32(np.iinfo(np.int32).max)
OUT_ROWS = N * 27

_NC_CACHE = {}


def _build_nc():
    if "nc" in _NC_CACHE:
        return _NC_CACHE["nc"]
    import concourse.bass as bass
    import concourse.bacc as bacc
    import concourse.tile as tile
    from concourse import mybir

    u8 = mybir.dt.uint8
    i32 = mybir.dt.int32
    MAX = mybir.AluOpType.max

    MUL = mybir.AluOpType.mult
    ADD = mybir.AluOpType.add
    nc = bacc.Bacc("TRN2", target_bir_lowering=False, num_devices=8)
    idxin = nc.dram_tensor("idxin", [128, IDX_COLS], i32, kind="ExternalInput")
    dil = nc.dram_tensor("dil", [ZPL * 260, 33], u8, kind="ExternalOutput")
    grid = nc.dram_tensor("grid", [GRID_CELLS, 1], u8)

    with tile.TileContext(nc) as tc:
        with (
            tc.tile_pool(name="sbuf", bufs=2) as pool,
            tc.tile_pool(name="ld", bufs=8) as ldp,
        ):
            # ---- zero the occupancy grid ----
            z8 = pool.tile([128, 8192], u8, tag="z8")
            nc.vector.memset(z8[:], 0)
            CH = 128 * 8192
            nfull = GRID_CELLS // CH
            for i in range(nfull):
                nc.sync.dma_start(
                    out=bass.AP(grid, i * CH, [[8192, 128], [1, 8192]]),
                    in_=z8[:],
                )
            remc = (GRID_CELLS - nfull * CH) // 128
            nc.sync.dma_start(
                out=bass.AP(grid, nfull * CH, [[remc, 128], [1, remc]]),
                in_=z8[:, :remc],
            )
            # ---- load indices, scatter occupancy ----
            idxsb = pool.tile([128, IDX_COLS], i32, tag="idx")
            nc.scalar.dma_start(out=idxsb[:], in_=idxin[:, :])
            ones = pool.tile([128, IDX_COLS], u8, tag="ones")
            nc.vector.memset(ones[:], 1)
            tc.strict_bb_all_engine_barrier()
            SW = 1  # descriptor batch width per indirect DMA (SW>1 mis-pairs offsets)
            for w in range(0, IDX_COLS, SW):
                nc.gpsimd.indirect_dma_start(
                    out=grid[:, :],
                    out_offset=bass.IndirectOffsetOnAxis(ap=idxsb[:, w:w + SW], axis=0),
                    in_=ones[:, w:w + SW],
                    in_offset=None,
                    bounds_check=GRID_CELLS - 1,
                    oob_is_err=False,
                )
            tc.strict_bb_all_engine_barrier()
            # ---- dilation ----
            engs = [nc.sync, nc.scalar]
            li = 0
            for zi in range(ZPL):
                for (r0, nrows) in ((0, 128), (128, 128), (256, 4)):
                    acc = ldp.tile([128, 512], u8, tag="acc")
                    first = True
                    for dz in (0, 2, 4):
                        for dy in (-2, 0, 2):
                            rs = r0 + dy
                            s = max(0, rs)
                            e = min(260, rs + nrows)
                            tmp = ldp.tile([128, 512], u8, tag=f"tmp{li % 4}")
                            if e - s < nrows:
                                nc.vector.memset(tmp[:nrows], 0)
                            off = (zi + dz) * PLANE + s * 512
                            engs[li % 2].dma_start(
                                out=tmp[s - rs:s - rs + (e - s), :],
                                in_=bass.AP(grid, off, [[512, e - s], [1, 512]]),
                            )
                            li += 1
                            if first:
                                nc.vector.tensor_copy(acc[:nrows], tmp[:nrows])
                                first = False
                            else:
                                nc.vector.tensor_tensor(
                                    out=acc[:nrows], in0=acc[:nrows],
                                    in1=tmp[:nrows], op=MAX)
                    fin = ldp.tile([128, 512], u8, tag="fin")
                    nc.vector.tensor_copy(fin[:nrows], acc[:nrows])
                    nc.vector.tensor_tensor(
                        out=fin[:nrows, 0:510], in0=fin[:nrows, 0:510],
                        in1=acc[:nrows, 2:512], op=MAX)
                    nc.vector.tensor_tensor(
                        out=fin[:nrows, 2:512], in0=fin[:nrows, 2:512],
                        in1=acc[:nrows, 0:510], op=MAX)
                    # bit-pack cells 0..263 -> 33 bytes/row (little bit order)
                    p1 = ldp.tile([128, 132], u8, tag="p1")
                    nc.vector.scalar_tensor_tensor(
                        out=p1[:nrows], in0=fin[:nrows, 1:264:2], scalar=2,
                        in1=fin[:nrows, 0:264:2], op0=MUL, op1=ADD)
                    p2 = ldp.tile([128, 66], u8, tag="p2")
                    nc.vector.scalar_tensor_tensor(
                        out=p2[:nrows], in0=p1[:nrows, 1:132:2], scalar=4,
                        in1=p1[:nrows, 0:132:2], op0=MUL, op1=ADD)
                    p3 = ldp.tile([128, 33], u8, tag="p3")
                    nc.vector.scalar_tensor_tensor(
                        out=p3[:nrows], in0=p2[:nrows, 1:66:2], scalar=16,
                        in1=p2[:nrows, 0:66:2], op0=MUL, op1=ADD)
                    nc.sync.dma_start(
                        out=dil[zi * 260 + r0: zi * 260 + r0 + nrows, :],
                        in_=p3[:nrows, :],
                    )
    nc.compile()
    _NC_CACHE["nc"] = nc
    return nc


def _shard_inputs(coords):
    zp = coords[:, 0].astype(np.int64) + 2
    yp = coords[:, 1].astype(np.int64) + 2
    xp = coords[:, 2].astype(np.int64) + 2
    in_maps = []
    for c in range(8):
        lo = 33 * c - 2
        sel = (zp >= lo) & (zp < lo + GRIDP)
        idx = ((zp[sel] - lo) * PLANE + yp[sel] * 512 + xp[sel]).astype(np.int32)
        if idx.size > NIDX:
            raise ValueError(f"core {c}: {idx.size} coords exceed capacity {NIDX}")
        pad = np.full(NIDX, 0x7FFF0000, np.int32)
        pad[:idx.size] = idx
        in_maps.append({"idxin": np.ascontiguousarray(pad.reshape(IDX_COLS, 128).T)})
    return in_maps


_LAST_TIMES = {}


def kernel(coords, stride):
    import time as _time
    from concourse.bass_utils import run_bass_kernel_spmd

    coords = np.asarray(coords)
    stride = int(np.asarray(stride))
    assert stride == 2, f"kernel hardcodes stride 2, got {stride}"
    assert coords.shape == (N, 4)

    t0 = _time.time()
    nc = _build_nc()
    t1 = _time.time()
    in_maps = _shard_inputs(coords)
    t2 = _time.time()
    res = run_bass_kernel_spmd(nc, in_maps, core_ids=list(range(8)))
    t3 = _time.time()
    _LAST_TIMES.update(build=t1 - t0, shard=t2 - t1, device=t3 - t2)

    from concurrent.futures import ThreadPoolExecutor

    def _extract(c):
        npl = min(ZPL, 260 - ZPL * c)
        packed = np.asarray(res.results[c]["dil"])[: npl * 260, :]
        # bits 260..263 of each 264-wide unpacked row are provably never set
        # (occupancy x <= 257, +-2 dilation reach <= 259), so flatnonzero can
        # run on the padded width directly; keys live in 264-stride space.
        bits = np.unpackbits(packed, axis=1, bitorder="little").reshape(-1)
        return np.flatnonzero(bits).astype(np.int32) + np.int32(ZPL * c * (260 * 264))

    with ThreadPoolExecutor(8) as ex:
        keys = list(ex.map(_extract, range(8)))
    keys = np.concatenate(keys)
    total = keys.size
    out = np.empty((OUT_ROWS, 4), np.int32)
    r, x = np.divmod(keys, np.int32(264))
    zq, y = np.divmod(r, np.int32(260))
    body = out[:total]
    body[:, 0] = zq
    body[:, 1] = y
    body[:, 2] = x
    body[:, 0:3] -= np.int32(2)
    body[:, 3] = 0
    out[total:] = FILL
    return out
